# revision 16
# baseline (speedup 1.0000x reference)
"""Packed causal GQA attention (B=4 x S=1024, H=32, KVH=8, D=DV=128, fp32)
for 8 Trainium2 NeuronCores.

Sharding: tensor-parallel over KV heads. Core c owns kv head c and its GQA
group of 4 query heads (4c..4c+3). No cross-core communication. Host-side
glue pre-transposes Q and K to [d, t] fp16 and casts V to fp16; the kernel
emits per-head-transposed out^T[dv, q] in fp16, which the host transposes
back while unsharding.

Key structure (v3):
  - Softmax denominator: P^T tiles are summed over k-blocks into
    lsum[128, 1024] (DVE fp16 2x adds; the two smallest k-blocks add on
    gpsimd), then one small 1024-col ones-matmul per (b,h) broadcasts
    l over partitions. PE streams ~2.07x of the causal P size instead of
    the 3x a full-size ones-matmul costs.
  - Score tiles pack k-blocks {3,6,7} and {4,5} into shared PSUM tiles so
    exp runs as 5 ACT instructions per unit instead of 8, cutting the
    352-cycle-per-instruction ACT pipeline-fill overhead.
  - Input DMAs are staggered (kT+q0 first, each further q head and V
    later) so the first QK matmul only waits on 0.5MB, not all of b=0.
  - Output is stored fp16 (halves output DMA).

Per-core pipeline, software-pipelined over 16 (b, h) units:
  front(u): S^T[k, q] = K^T.T @ Q^T per kb (fp16, causal column ranges,
    PSUM fp32, bank-aligned segments); P^T = Exp(SCALE*S^T) on ACT (one
    instruction per score tile; optional SCHR_SPLIT tail columns use a
    DVE Schraudolph exp); gpsimd affine_select zeroes the strictly-upper
    triangle of each diagonal block; lsum accumulates P^T over k-blocks.
  back(u): l_bcast = ones^T @ lsum (small matmul, broadcasts the
    denominator over 128 partitions); rsb = 1/l on DVE; out^T[dv, q] =
    sum_kb V[kb].T @ P^T[kb] in PSUM; out = out^T * rsb -> fp16; DMA out.
"""

import math
from collections import deque

import numpy as np

import concourse.bacc as bacc
import concourse.tile as tile
from concourse import mybir, bass_utils

T = 4096          # packed tokens
SEQ = 1024        # per-sequence length
B = T // SEQ      # 4 sequences
H = 32            # query heads (total)
KVH = 8           # kv heads (total)
D = 128           # head size
DV = 128          # value head size
NCORES = 8
HPC = H // NCORES         # 4 query heads per core
NB = SEQ // 128           # 8 k-blocks per sequence
SCALE = 0.08838834764831845

F16 = mybir.dt.float16
F32 = mybir.dt.float32
I16 = mybir.dt.int16

# kb -> (group_tag, local column offset of q=qs within the group tile, qs)
# qs = 128*kb is the first valid q column; q maps to tile col off + (q - qs).
KB_LAYOUT = {
    0: ("g0", 0, 0),
    1: ("g1", 0, 128),
    2: ("g2", 0, 256),
    3: ("g3x", 0, 384),
    4: ("g45", 0, 512),
    5: ("g45", 512, 640),
    6: ("g3x", 640, 768),
    7: ("g3x", 896, 896),
}
GROUPS = [("g0", (0,), 1024), ("g1", (1,), 896), ("g2", (2,), 768),
          ("g3x", (3, 6, 7), 1024), ("g45", (4, 5), 896)]
# k-blocks folded into the denominator via accumulating ones-matmuls on the
# PE instead of DVE adds into lsum (cheaper per column on the PE, and it
# shortens the serial DVE add chain)
PE_SUM_KBS = (7,)

# DVE-exp (Schraudolph) split: kb -> first q column handled by DVE instead
# of ACT (must be >= 128*kb + 128 so diagonal blocks stay exact).
SCHR_SPLIT = {}

# Schraudolph constants for fp16 target: bits = round(x*SA + SC),
# v = 2^((bits-15360)/1024) ~= exp(x). SC includes the -62 minimax shift.
SA = 1024.0 * SCALE / math.log(2.0)
SC = 15360.0 - 62.0

_BUILD_CACHE = {}


def _bank_segments(lo, hi):
    """Split local column range [lo, hi) at 512-col PSUM bank boundaries."""
    segs = []
    x = lo
    while x < hi:
        nxt = min(hi, (x // 512 + 1) * 512)
        segs.append((x, nxt))
        x = nxt
    return segs


def _build_nc():
    nc = bacc.Bacc("TRN2", target_bir_lowering=False, debug=False,
                   num_devices=NCORES)
    # host-pretransposed, fp16: qT[h*128+d, t], kT[d, t], v[t, dv]
    qt_dram = nc.dram_tensor("qT", [HPC * D, T], F16, kind="ExternalInput").ap()
    kt_dram = nc.dram_tensor("kT", [D, T], F16, kind="ExternalInput").ap()
    v_dram = nc.dram_tensor("v", [T, DV], F16, kind="ExternalInput").ap()
    # out_t[b*HPC + h, dv, q]  (transposed per-head output; host untransposes)
    out_dram = nc.dram_tensor("out_t", [B * HPC, DV, SEQ], F16,
                              kind="ExternalOutput").ap()

    with tile.TileContext(nc) as tc:
        with tc.tile_pool(name="consts", bufs=1) as consts, \
             tc.tile_pool(name="kv", bufs=2) as kv_pool, \
             tc.tile_pool(name="qts", bufs=5) as qt_pool, \
             tc.tile_pool(name="pt", bufs=2) as pt_pool, \
             tc.tile_pool(name="ls", bufs=2) as ls_pool, \
             tc.tile_pool(name="work", bufs=2) as work, \
             tc.tile_pool(name="pp_s", bufs=3, space="PSUM") as pp_s, \
             tc.tile_pool(name="pp_lo", bufs=1, space="PSUM") as pp_lo:

            ones_sb = consts.tile([128, 128], F16, tag="ones")
            nc.vector.memset(ones_sb[:], 1.0)

            kts = {}
            vs = {}
            qts = {}

            def load_kt(b):
                cols = slice(b * SEQ, (b + 1) * SEQ)
                kt = kv_pool.tile([128, NB, 128], F16, tag="kt")
                nc.sync.dma_start(
                    kt[:], kt_dram[:, cols].rearrange("d (nb t) -> d nb t", t=128))
                kts[b] = kt

            def load_qt(b, h):
                cols = slice(b * SEQ, (b + 1) * SEQ)
                qt = qt_pool.tile([128, NB, 128], F16, tag="qt")
                nc.sync.dma_start(
                    qt[:],
                    qt_dram[h * D:(h + 1) * D, cols].rearrange(
                        "d (nb t) -> d nb t", t=128))
                qts[(b, h)] = qt

            def load_v(b):
                rows = slice(b * SEQ, (b + 1) * SEQ)
                v_sb = kv_pool.tile([128, NB, DV], F16, tag="v")
                nc.sync.dma_start(
                    v_sb[:], v_dram[rows, :].rearrange("(nb p) d -> p nb d", p=128))
                vs[b] = v_sb

            def emit_front(b, h):
                """QK matmuls + exp + causal mask + lsum accumulation.

                Returns (pts, lsum) where pts maps group tag -> P^T tile."""
                kt = kts[b]
                qt = qts[(b, h)]
                pts = {}
                lsum = ls_pool.tile([128, SEQ], F16, tag="lsum")
                for gtag, kbs, gcols in GROUPS:
                    ps = pp_s.tile([128, 1024], F32, tag="ps")
                    pt = pt_pool.tile([128, gcols], F16, tag=gtag)
                    pts[gtag] = pt
                    for kb in kbs:
                        _, off, qs = KB_LAYOUT[kb]
                        for slo, shi in _bank_segments(off, off + SEQ - qs):
                            qlo = qs + slo - off
                            qhi = qs + shi - off
                            nc.tensor.matmul(
                                ps[:, slo:shi],
                                kt[:, kb, :],
                                qt[:, qlo // 128:qhi // 128, :],
                                start=True, stop=True, skip_group_check=True)
                    # exp: ACT on head columns, DVE Schraudolph on tails
                    act_ranges = []   # tile-local (lo, hi) for ACT
                    dve_ranges = []
                    for kb in kbs:
                        _, off, qs = KB_LAYOUT[kb]
                        cols = SEQ - qs
                        split_q = SCHR_SPLIT.get(kb, SEQ)
                        split_loc = off + max(128, min(split_q, SEQ) - qs)
                        split_loc = min(split_loc, off + cols)
                        if split_loc > off:
                            act_ranges.append([off, split_loc])
                        if split_loc < off + cols:
                            dve_ranges.append((split_loc, off + cols))
                    merged = []
                    for lo, hi in act_ranges:
                        if merged and merged[-1][1] == lo:
                            merged[-1][1] = hi
                        else:
                            merged.append([lo, hi])
                    for lo, hi in merged:
                        nc.scalar.activation(
                            pt[:, lo:hi], ps[:, lo:hi],
                            mybir.ActivationFunctionType.Exp, scale=SCALE)
                    for lo, hi in dve_ranges:
                        nc.vector.tensor_scalar(
                            out=pt[:, lo:hi].bitcast(I16),
                            in0=ps[:, lo:hi],
                            scalar1=SA, scalar2=SC,
                            op0=mybir.AluOpType.mult, op1=mybir.AluOpType.add)
                    # zero strictly-upper triangle of each diagonal block
                    for kb in kbs:
                        _, off, qs = KB_LAYOUT[kb]
                        nc.gpsimd.affine_select(
                            out=pt[:, off:off + 128], in_=pt[:, off:off + 128],
                            compare_op=mybir.AluOpType.is_ge,
                            fill=0.0, base=0,
                            pattern=[[1, 128]], channel_multiplier=-1)
                # accumulate lsum over k-blocks (kb order; serial on lsum);
                # PE_SUM_KBS skip lsum and join via ones-matmuls in back().
                # lsum[0:128] stays UNWRITTEN: pt0[:, 0:128] joins via its
                # own ones-matmul in back() instead of a DVE copy.
                pt0 = pts["g0"]
                pt1 = pts["g1"]
                nc.vector.tensor_tensor(
                    out=lsum[:, 128:SEQ], in0=pt0[:, 128:SEQ],
                    in1=pt1[:, 0:896], op=mybir.AluOpType.add)
                for kb in range(2, NB):
                    if kb in PE_SUM_KBS:
                        continue
                    gtag, off, qs = KB_LAYOUT[kb]
                    nc.vector.tensor_tensor(
                        out=lsum[:, qs:SEQ], in0=lsum[:, qs:SEQ],
                        in1=pts[gtag][:, off:off + SEQ - qs],
                        op=mybir.AluOpType.add)
                return pts, lsum

            def emit_back(b, h, pts, lsum):
                """denominator broadcast + PV matmuls + normalize + store."""
                v_sb = vs[b]
                l_ps = pp_lo.tile([128, SEQ], F32, tag="lo")
                # bank 0: pt0's first block (start=True, sets has_written on
                # [0:128] only) then lsum[128:512] lands in overwrite mode.
                nc.tensor.matmul(
                    l_ps[:, 0:128], ones_sb[:], pts["g0"][:, 0:128],
                    start=True, stop=False, skip_group_check=True)
                nc.tensor.matmul(
                    l_ps[:, 128:512], ones_sb[:], lsum[:, 128:512],
                    start=False, stop=True, skip_group_check=True)
                # bank 1: lsum plus the PE_SUM_KBS tiles accumulate
                # (all PE_SUM_KBS have qs >= 512 so they land in bank 1)
                nc.tensor.matmul(
                    l_ps[:, 512:SEQ], ones_sb[:], lsum[:, 512:SEQ],
                    start=True, stop=False, skip_group_check=True)
                for i, kb in enumerate(PE_SUM_KBS):
                    gtag, off, qs = KB_LAYOUT[kb]
                    assert qs >= 512
                    nc.tensor.matmul(
                        l_ps[:, qs:SEQ], ones_sb[:],
                        pts[gtag][:, off:off + SEQ - qs],
                        start=False, stop=(i == len(PE_SUM_KBS) - 1),
                        skip_group_check=True)
                rsb = work.tile([128, SEQ], F32, tag="rsb")
                nc.vector.reciprocal_approx_fast(rsb[:], l_ps[:])
                ps_o = pp_lo.tile([128, SEQ], F32, tag="lo")
                for qc in range(2):
                    kbs = list(range(0, 4 * qc + 4))
                    for kb in kbs:
                        gtag, off, qs0 = KB_LAYOUT[kb]
                        s = max(128 * kb, 512 * qc)
                        e = 512 * (qc + 1)
                        rhs = pts[gtag][:, off + s - qs0:off + e - qs0]
                        nc.tensor.matmul(
                            ps_o[:, s:e], v_sb[:, kb, :], rhs,
                            start=(kb == 0), stop=(kb == kbs[-1]),
                            skip_group_check=True)
                out_sb = work.tile([128, SEQ], F16, tag="out_sb")
                nc.vector.tensor_tensor(
                    out=out_sb[:], in0=ps_o[:], in1=rsb[:],
                    op=mybir.AluOpType.mult)
                nc.sync.dma_start(out_dram[b * HPC + h], out_sb[:])

            # software-pipelined emission: front(u+1) before back(u) so the
            # PE's static instruction order interleaves the next unit's QK
            # matmuls with the previous unit's PV matmuls. Input DMAs are
            # staggered: kT+q0 first so the first matmul starts early.
            units = [(b, h) for b in range(B) for h in range(HPC)]
            pending = deque()   # (b, h, pts, lsum)
            for b, h in units:
                if h == 0:
                    load_kt(b)
                    load_qt(b, 0)
                pts, lsum = emit_front(b, h)
                if h == 0:
                    load_qt(b, 1)
                    load_v(b)
                elif h < HPC - 1:
                    load_qt(b, h + 1)
                pending.append((b, h, pts, lsum))
                if len(pending) > 1:
                    emit_back(*pending.popleft())
            while pending:
                emit_back(*pending.popleft())

    nc.compile()
    return nc


def run_sharded(query, key, value, trace=False):
    """Shard over 8 cores, run the bass kernel, unshard. Returns
    (out [T, H*DV] fp32, BassKernelResults)."""
    query = np.asarray(query, dtype=np.float32)
    key = np.asarray(key, dtype=np.float32)
    value = np.asarray(value, dtype=np.float32)

    if "nc" not in _BUILD_CACHE:
        _BUILD_CACHE["nc"] = _build_nc()
    nc = _BUILD_CACHE["nc"]

    # host layout glue: cast to fp16, then transpose to [d, t]
    qT = np.ascontiguousarray(query.astype(np.float16).T)   # [H*D, T]
    kT = np.ascontiguousarray(key.astype(np.float16).T)     # [KVH*D, T]
    v16 = np.ascontiguousarray(value.astype(np.float16))    # [T, KVH*DV]

    in_maps = []
    for c in range(NCORES):
        in_maps.append({
            "qT": np.ascontiguousarray(qT[c * HPC * D:(c + 1) * HPC * D]),
            "kT": np.ascontiguousarray(kT[c * D:(c + 1) * D]),
            "v": np.ascontiguousarray(v16[:, c * DV:(c + 1) * DV]),
        })

    res = bass_utils.run_bass_kernel_spmd(
        nc, in_maps, core_ids=list(range(NCORES)), trace=trace)

    outs = []
    for c in range(NCORES):
        ot = res.results[c]["out_t"].astype(np.float32)     # [B*HPC, DV, SEQ]
        o = ot.reshape(B, HPC, DV, SEQ).transpose(0, 3, 1, 2).reshape(T, HPC * DV)
        outs.append(o)
    return np.concatenate(outs, axis=1), res


def kernel(query, key, value, seq_len=1024, **_unused):
    assert int(seq_len) == SEQ, f"kernel hardcodes seq_len={SEQ}, got {seq_len}"
    out, _ = run_sharded(query, key, value, trace=False)
    return out
